# revision 18
# baseline (speedup 1.0000x reference)
"""Trainium2 Bass kernel for AttnDecoderGRU single step (8-core SPMD).

Shapes (hardcoded): H1=512, H=2048, OUT=1024, T=4096, 8 cores.
Sharding: T-shard encoder_outputs (512 rows/core, kept transposed h-major),
GRU gate rows sharded 8-way (256 rows of each of r/z/n per core),
attn1 column-sharded with an AllGather of `way`; attention exchange and
final logits/h_new exchange via small AllGathers.
"""

import os
import numpy as np

os.environ.setdefault("MYCRO_LOCAL_CACHE", "1")

H1, H, OUT, T = 512, 2048, 1024, 4096
NCORES = 8
P = 128
TLOC = T // NCORES          # 512 enc rows per core
GSL = H // NCORES           # 256 gate rows per core (per r/z/n block)
G3 = 3 * GSL                # 768 gate rows total per core

# attention-exchange piece layout (fp32 elements)
PC_E = TLOC                 # exp(gamma) piece            [0:512)
PC_AT = H                   # attn_appliedT partial       [512:2560)
PIECE = 2592                # e | aT | S | pad  (32B-aligned)
# final-exchange piece layout
FPIECE = OUT + GSL          # logits partial | h_new slice


# dtype knobs: "f32" | "f32r" | "bf16" for the attention pair (encT/a2T),
# and "f32" | "bf16" for the weight matrices (prenet/attn1/gru/lin).
DT_ATT = os.environ.get("KDT_ATT", "bf16")
DT_W = os.environ.get("KDT_W", "bf16")

_CACHE = {}


def _build():
    import concourse.bass as bass
    import concourse.bacc as bacc
    import concourse.mybir as mybir
    import concourse.tile as tile

    f32 = mybir.dt.float32
    AF = mybir.ActivationFunctionType
    OpT = mybir.AluOpType

    nc = bacc.Bacc(None)

    bf16 = mybir.dt.bfloat16
    datt = {"f32": f32, "f32r": mybir.dt.float32r, "bf16": bf16}[DT_ATT]
    dw = {"f32": f32, "bf16": bf16}[DT_W]
    # beta/gamma operand dtype: bf16 whenever the big matmul is reduced-prec
    dbeta = f32 if DT_ATT == "f32" else bf16

    def inp(name, shape, dtype=f32):
        return nc.dram_tensor(name, shape, dtype, kind="ExternalInput")

    encT = inp("encT", [H, TLOC], datt)
    a2T = inp("a2T", [H, H], datt)
    a1T_s = inp("a1T_s", [H, GSL], dw)
    a3c = inp("a3c", [P, H // P], dbeta)
    pwT = inp("pwT", [OUT, H1], dw)
    p2w = inp("p2w", [H, H1], dw)
    p2b_c = inp("p2b_c", [P, H // P])
    pb_row = inp("pb_row", [1, H1])
    in_c = inp("in_c", [P, OUT // P], dw)
    h0c = inp("h0c", [P, H // P], dw)
    h0sl = inp("h0sl", [1, GSL])
    wihT = inp("wihT", [2 * H, G3], dw)
    whhT = inp("whhT", [H, G3], dw)
    bih_r = inp("bih_r", [1, G3])
    bhh_r = inp("bhh_r", [1, G3])
    linT_s = inp("linT_s", [GSL, OUT], dw)
    linb8 = inp("linb8", [1, OUT])
    ones11 = inp("ones11", [1, 1])
    ones_row = inp("ones_row", [1, TLOC])

    out_o = nc.dram_tensor("out", [1, OUT], f32, kind="ExternalOutput")
    out_h = nc.dram_tensor("h_new", [1, H], f32, kind="ExternalOutput")
    out_aw = nc.dram_tensor("attn_weights", [1, T], f32, kind="ExternalOutput")

    RG = [list(range(NCORES))]
    KH = H // P   # 16 k-tiles over hidden dim


    with tile.TileContext(nc) as tc:
        with (
            tc.tile_pool(name="enc", bufs=1) as enc_pool,
            tc.tile_pool(name="w2", bufs=3) as w2_pool,
            tc.tile_pool(name="gruw", bufs=4) as gruw_pool,
            tc.tile_pool(name="linw", bufs=1) as linw_pool,
            tc.tile_pool(name="pn", bufs=2) as pn_pool,
            tc.tile_pool(name="sm", bufs=1) as sm_pool,
            tc.tile_pool(name="scr", bufs=2) as scr_pool,
            tc.tile_pool(name="beta", bufs=3) as beta_pool,
            tc.tile_pool(name="psA", bufs=2, space="PSUM") as psA,
            tc.tile_pool(name="psG", bufs=1, space="PSUM") as psG,
            tc.tile_pool(name="psV", bufs=4, space="PSUM") as psV,
            tc.tile_pool(name="psL", bufs=1, space="PSUM") as psL,
            tc.tile_pool(name="dram", bufs=1, space="DRAM") as dram,
        ):
            # ------------- small-constant SBUF loads -------------
            in_sb = sm_pool.tile([P, OUT // P], dw)
            nc.sync.dma_start(in_sb, in_c[:])
            h0c_sb = sm_pool.tile([P, KH], dw)
            nc.sync.dma_start(h0c_sb, h0c[:])
            a3_sb = sm_pool.tile([P, KH], dbeta)
            nc.sync.dma_start(a3_sb, a3c[:])
            pb_sb = sm_pool.tile([1, H1], f32)
            nc.sync.dma_start(pb_sb, pb_row[:])
            bih_sb = sm_pool.tile([1, G3], f32)
            nc.sync.dma_start(bih_sb, bih_r[:])
            bhh_sb = sm_pool.tile([1, G3], f32)
            nc.sync.dma_start(bhh_sb, bhh_r[:])
            lb8_sb = sm_pool.tile([1, OUT], f32)
            nc.sync.dma_start(lb8_sb, linb8[:])
            one_sb = sm_pool.tile([1, 1], f32)
            nc.sync.dma_start(one_sb, ones11[:])
            onesr_sb = sm_pool.tile([1, TLOC], f32)
            nc.sync.dma_start(onesr_sb, ones_row[:])
            pwT_sb = sm_pool.tile([P, OUT // P, H1], dw)
            nc.sync.dma_start(pwT_sb, pwT.rearrange("(k p) n -> p k n", p=P))
            a1_sb = sm_pool.tile([P, KH, GSL], dw)
            nc.sync.dma_start(a1_sb, a1T_s.rearrange("(k p) n -> p k n", p=P))

            # ---------------- prenet ----------------
            # em1 = relu(input @ prenet_w.T + pb)   row [1, 512]
            em1_ps = psV.tile([1, H1], f32, tag="vec")
            for k in range(OUT // P):
                nc.tensor.matmul(
                    em1_ps, lhsT=in_sb[:, k : k + 1],
                    rhs=pwT_sb[:, k],
                    start=(k == 0), stop=False)
            nc.tensor.matmul(em1_ps, lhsT=one_sb[:], rhs=pb_sb[:],
                             start=False, stop=True)
            em1_row = sm_pool.tile([1, H1], f32)
            nc.scalar.activation(em1_row, em1_ps, AF.Relu)
            em1_b = sm_pool.tile([P, H1], f32)
            nc.gpsimd.partition_broadcast(em1_b, em1_row, channels=P)

            # em6T column [128, 16] via DVE dot products on natural prenet2_w
            em6_pre = sm_pool.tile([P, KH], f32)
            for m in range(KH):
                p2w_t = pn_pool.tile([P, H1], dw, tag="pn")
                nc.sync.dma_start(
                    p2w_t, p2w.rearrange("(m p) n -> p m n", p=P)[:, m])
                scr = scr_pool.tile([P, H1], f32, tag="scr")
                nc.vector.tensor_tensor(scr, p2w_t, em1_b, OpT.mult)
                nc.vector.tensor_reduce(
                    em6_pre[:, m : m + 1], scr,
                    axis=mybir.AxisListType.X, op=OpT.add)
            # + prenet2 bias (column layout) then relu
            p2b_sb = sm_pool.tile([P, KH], f32)
            nc.sync.dma_start(p2b_sb, p2b_c[:])
            em6c = sm_pool.tile([P, KH], f32)
            nc.vector.tensor_tensor(em6c, em6_pre, p2b_sb, OpT.add)
            nc.vector.tensor_scalar_max(em6c, em6c, 0.0)
            if dw != f32:
                em6m = sm_pool.tile([P, KH], dw)
                nc.vector.tensor_copy(em6m, em6c)
            else:
                em6m = em6c

            # ---------------- way (sharded col-slice + AllGather) -------
            way_ps = psV.tile([1, GSL], f32, tag="vec")
            for k in range(KH):
                nc.tensor.matmul(
                    way_ps, lhsT=em6m[:, k : k + 1],
                    rhs=a1_sb[:, k],
                    start=(k == 0), stop=(k == KH - 1))
            way_piece = sm_pool.tile([1, GSL], f32)
            nc.vector.tensor_copy(way_piece, way_ps)
            way_in = dram.tile([1, GSL], f32, name="way_in")
            nc.sync.dma_start(way_in[:], way_piece)
            way_out = dram.tile([1, H], f32, name="way_out",
                                addr_space="Shared")
            nc.gpsimd.collective_compute(
                "AllGather", OpT.bypass, replica_groups=RG,
                ins=[way_in.opt()], outs=[way_out.opt()])
            way_row = sm_pool.tile([1, H], f32)
            nc.sync.dma_start(way_row, way_out[:])

            # ---------------- encoder load ----------------
            encT_t = enc_pool.tile([P, KH, TLOC], datt)
            nc.sync.dma_start(encT_t, encT.rearrange("(k p) t -> p k t", p=P))

            # ---------------- big matmul: betaT & gamma ----------------
            # bf16: whole a2T resident (8.4MB, clean row-block DMAs).
            # fp32/f32r: stream column blocks (doesn't fit resident).
            resident_a2 = DT_ATT == "bf16"
            if resident_a2:
                a2_full = enc_pool.tile([P, KH, H], datt, name="a2_full")
                for k in range(KH):
                    nc.sync.dma_start(
                        a2_full[:, k],
                        a2T[k * P : (k + 1) * P, :])
            # rank-1 way operands in matmul dtype
            if dbeta != f32:
                way_m = sm_pool.tile([1, H], dbeta)
                nc.vector.tensor_copy(way_m, way_row)
                onesr_m = sm_pool.tile([1, TLOC], dbeta)
                nc.vector.tensor_copy(onesr_m, onesr_sb)
            else:
                way_m, onesr_m = way_row, onesr_sb
            gam_ps = psG.tile([1, TLOC], f32)
            for m in range(KH):
                if resident_a2:
                    a2_t = a2_full[:, :, m * P : (m + 1) * P]
                else:
                    a2_t = w2_pool.tile([P, KH, P], datt, tag="a2")
                    nc.sync.dma_start(
                        a2_t,
                        a2T.rearrange("(k p) c -> p k c", p=P)[
                            :, :, m * P : (m + 1) * P])
                ps = psA.tile([P, TLOC], f32, tag="uah")
                for k in range(KH):
                    nc.tensor.matmul(ps, lhsT=a2_t[:, k],
                                     rhs=encT_t[:, k],
                                     start=(k == 0), stop=False)
                # + way broadcast along t (rank-1)
                nc.tensor.matmul(ps, lhsT=way_m[:, m * P : (m + 1) * P],
                                 rhs=onesr_m[:], start=False, stop=True)
                betaT_m = beta_pool.tile([P, TLOC], dbeta, tag="beta")
                nc.scalar.activation(betaT_m, ps, AF.Tanh)
                nc.tensor.matmul(gam_ps, lhsT=a3_sb[:, m : m + 1],
                                 rhs=betaT_m,
                                 start=(m == 0), stop=(m == KH - 1))

            # -------- softmax pieces (no max-sub; gamma is O(1)-bounded)
            e_row = sm_pool.tile([1, TLOC], f32)
            s_part = sm_pool.tile([1, 1], f32)
            nc.scalar.activation(e_row, gam_ps, AF.Exp, accum_out=s_part)
            e_b = sm_pool.tile([P, TLOC], f32)
            nc.gpsimd.partition_broadcast(e_b, e_row, channels=P)

            # attn_appliedT partial via DVE dot products on resident encT
            aT_pre = sm_pool.tile([P, KH], f32)
            for k in range(KH):
                scr2 = scr_pool.tile([P, TLOC], f32, tag="scr")
                enc_view = (encT_t[:, k].bitcast(f32)
                            if DT_ATT == "f32r" else encT_t[:, k])
                nc.vector.tensor_tensor(scr2, enc_view, e_b, OpT.mult)
                nc.vector.tensor_reduce(
                    aT_pre[:, k : k + 1], scr2,
                    axis=mybir.AxisListType.X, op=OpT.add)

            # assemble AllGather piece: [e | aT | S | pad]
            att_in = dram.tile([1, PIECE], f32, name="att_in")
            zpad = sm_pool.tile([1, PIECE - (PC_E + PC_AT + 1)], f32)
            nc.vector.memset(zpad, 0.0)
            nc.sync.dma_start(att_in[:, 0:PC_E], e_row)
            nc.sync.dma_start(
                att_in[0, PC_E : PC_E + PC_AT].rearrange("(p k) -> p k", p=P),
                aT_pre)
            nc.sync.dma_start(
                att_in[:, PC_E + PC_AT : PC_E + PC_AT + 1], s_part)
            nc.sync.dma_start(att_in[:, PC_E + PC_AT + 1 :], zpad)
            att_out = dram.tile([NCORES, PIECE], f32, name="att_out",
                                addr_space="Shared")
            nc.gpsimd.collective_compute(
                "AllGather", OpT.bypass, replica_groups=RG,
                ins=[att_in.opt()], outs=[att_out.opt()])

            # ---- read back: e_all [128,32], aT sum over ranks, S sum ----
            e_all = sm_pool.tile([P, T // P], f32)
            JW = T // P  # 32 exp values per partition row
            for r in range(NCORES):
                nc.sync.dma_start(
                    e_all[r * (TLOC // JW) : (r + 1) * (TLOC // JW), :],
                    att_out[r, 0:PC_E].rearrange("(p j) -> p j", p=TLOC // JW))
            aT_rk = sm_pool.tile([P, NCORES, KH], f32)
            nc.sync.dma_start(
                aT_rk,
                att_out[:, PC_E : PC_E + PC_AT]
                .rearrange("r (p k) -> p r k", p=P))
            aT_full = sm_pool.tile([P, KH], f32)
            nc.vector.tensor_reduce(
                aT_full, aT_rk.rearrange("p r k -> p k r"),
                axis=mybir.AxisListType.X, op=OpT.add)
            s_rk = sm_pool.tile([1, NCORES], f32)
            nc.sync.dma_start(
                s_rk,
                att_out[:, PC_E + PC_AT : PC_E + PC_AT + 1]
                .rearrange("r n -> n r"))
            s_tot = sm_pool.tile([1, 1], f32)
            nc.vector.tensor_reduce(s_tot, s_rk, axis=mybir.AxisListType.X,
                                    op=OpT.add)
            rs = sm_pool.tile([1, 1], f32)
            nc.vector.reciprocal(rs, s_tot)
            rs_b = sm_pool.tile([P, 1], f32)
            nc.gpsimd.partition_broadcast(rs_b, rs, channels=P)

            # attn_weights output
            w_sb = sm_pool.tile([P, T // P], f32)
            nc.vector.tensor_scalar_mul(w_sb, e_all, rs_b)
            nc.sync.dma_start(
                out_aw[0].rearrange("(p j) -> p j", p=P), w_sb)
            # normalized attn_appliedT column [128, 16]
            aT_n = sm_pool.tile([P, KH], dw)
            nc.vector.tensor_scalar_mul(aT_n, aT_full, rs_b)

            # ---------------- GRU gate matvecs ----------------
            NCH = [(0, 512), (512, 256)]  # n-chunks of the 768 gate rows

            def stream_w(src, kk):
                t_ = gruw_pool.tile([P, G3], dw, tag="gw")
                nc.sync.dma_start(
                    t_, src.rearrange("(k p) n -> p k n", p=P)[:, kk])
                return t_

            gi_ps = [psV.tile([1, n], f32, tag="vec", name=f"gi{i}")
                     for i, (_, n) in enumerate(NCH)]
            gh_ps = [psV.tile([1, n], f32, tag="vec", name=f"gh{i}")
                     for i, (_, n) in enumerate(NCH)]
            # hh: k 0..15 on h0
            for k in range(KH):
                w_t = stream_w(whhT, k)
                for i, (o, n) in enumerate(NCH):
                    nc.tensor.matmul(gh_ps[i], lhsT=h0c_sb[:, k : k + 1],
                                     rhs=w_t[:, o : o + n],
                                     start=(k == 0), stop=False)
            for i, (o, n) in enumerate(NCH):
                nc.tensor.matmul(gh_ps[i], lhsT=one_sb[:],
                                 rhs=bhh_sb[:, o : o + n],
                                 start=False, stop=True)
            # ih: k 0..15 em6 part, k 16..31 attn part
            for k in range(KH):
                w_t = stream_w(wihT, k)
                for i, (o, n) in enumerate(NCH):
                    nc.tensor.matmul(gi_ps[i], lhsT=em6m[:, k : k + 1],
                                     rhs=w_t[:, o : o + n],
                                     start=(k == 0), stop=False)
            for k in range(KH):
                w_t = stream_w(wihT, KH + k)
                for i, (o, n) in enumerate(NCH):
                    nc.tensor.matmul(gi_ps[i], lhsT=aT_n[:, k : k + 1],
                                     rhs=w_t[:, o : o + n],
                                     start=False, stop=False)
            for i, (o, n) in enumerate(NCH):
                nc.tensor.matmul(gi_ps[i], lhsT=one_sb[:],
                                 rhs=bih_sb[:, o : o + n],
                                 start=False, stop=True)

            # ---------------- gates ----------------
            # rz = sigmoid(gi[0:512] + gh[0:512]) via 0.5*(1+tanh(x/2))
            gh0_sb = sm_pool.tile([1, 2 * GSL], f32)
            nc.vector.tensor_copy(gh0_sb, gh_ps[0])
            rz_pre = sm_pool.tile([1, 2 * GSL], f32)
            nc.vector.tensor_tensor(rz_pre, gi_ps[0], gh0_sb, OpT.add)
            t_rz = sm_pool.tile([1, 2 * GSL], f32)
            nc.scalar.activation(t_rz, rz_pre, AF.Tanh, scale=0.5)
            rz = sm_pool.tile([1, 2 * GSL], f32)
            nc.vector.tensor_scalar(rz, t_rz, 0.5, 0.5, OpT.mult, OpT.add)
            # n = tanh(gi_n + r * gh_n)
            n_pre = sm_pool.tile([1, GSL], f32)
            nc.vector.tensor_tensor(n_pre, rz[:, 0:GSL], gh_ps[1], OpT.mult)
            nc.vector.tensor_tensor(n_pre, n_pre, gi_ps[1], OpT.add)
            n_g = sm_pool.tile([1, GSL], f32)
            nc.scalar.activation(n_g, n_pre, AF.Tanh)
            # h_new_slice = n + z*(h0sl - n)
            h0_sb = sm_pool.tile([1, GSL], f32)
            nc.sync.dma_start(h0_sb, h0sl[:])
            t1 = sm_pool.tile([1, GSL], f32)
            nc.vector.tensor_tensor(t1, h0_sb, n_g, OpT.subtract)
            nc.vector.tensor_tensor(t1, rz[:, GSL : 2 * GSL], t1, OpT.mult)
            hn_sl = sm_pool.tile([1, GSL], f32)
            nc.vector.tensor_tensor(hn_sl, n_g, t1, OpT.add)

            # ---------------- h_newT column + logits partial ------------
            hnT_ps = psL.tile([P, GSL // P], f32)
            for i in range(GSL // P):
                nc.tensor.matmul(hnT_ps[:, i : i + 1],
                                 lhsT=hn_sl[:, i * P : (i + 1) * P],
                                 rhs=one_sb[:], start=True, stop=True)
            hnT = sm_pool.tile([P, GSL // P], dw)
            nc.vector.tensor_copy(hnT, hnT_ps)

            lin_t = linw_pool.tile([P, GSL // P, OUT], dw)
            nc.sync.dma_start(
                lin_t, linT_s.rearrange("(k p) n -> p k n", p=P))
            lg_ps = [psV.tile([1, 512], f32, tag="vec", name=f"lg{j}")
                     for j in range(2)]
            for j in range(2):
                for i in range(GSL // P):
                    nc.tensor.matmul(
                        lg_ps[j], lhsT=hnT[:, i : i + 1],
                        rhs=lin_t[:, i, j * 512 : (j + 1) * 512],
                        start=(i == 0), stop=False)
                nc.tensor.matmul(lg_ps[j], lhsT=one_sb[:],
                                 rhs=lb8_sb[:, j * 512 : (j + 1) * 512],
                                 start=False, stop=True)
            lg_row = sm_pool.tile([1, OUT], f32)
            nc.vector.tensor_copy(lg_row[:, 0:512], lg_ps[0])
            nc.vector.tensor_copy(lg_row[:, 512:1024], lg_ps[1])

            # ---------------- final AllGather: logits + h_new ----------
            fin_in = dram.tile([1, FPIECE], f32, name="fin_in")
            nc.sync.dma_start(fin_in[:, 0:OUT], lg_row)
            nc.sync.dma_start(fin_in[:, OUT:], hn_sl)
            fin_out = dram.tile([NCORES, FPIECE], f32, name="fin_out",
                                addr_space="Shared")
            nc.gpsimd.collective_compute(
                "AllGather", OpT.bypass, replica_groups=RG,
                ins=[fin_in.opt()], outs=[fin_out.opt()])

            # h_new output: gathered slices are rank-ordered = full h_new
            hn_full = sm_pool.tile([NCORES, GSL], f32)
            nc.sync.dma_start(hn_full, fin_out[:, OUT:])
            nc.sync.dma_start(out_h[0].rearrange("(r n) -> r n", r=NCORES),
                              hn_full)

            # logits: sum partials over ranks, softmax, write out
            lg_rk = sm_pool.tile([P, NCORES, OUT // P], f32)
            nc.sync.dma_start(
                lg_rk,
                fin_out[:, 0:OUT].rearrange("r (p j) -> p r j", p=P))
            lgt = sm_pool.tile([P, OUT // P], f32)
            nc.vector.tensor_reduce(
                lgt, lg_rk.rearrange("p r j -> p j r"),
                axis=mybir.AxisListType.X, op=OpT.add)
            e2 = sm_pool.tile([P, OUT // P], f32)
            s2 = sm_pool.tile([P, 1], f32)
            nc.scalar.activation(e2, lgt, AF.Exp, accum_out=s2)
            import concourse.bass_isa as bass_isa
            s2t = sm_pool.tile([P, 1], f32)
            nc.gpsimd.partition_all_reduce(s2t, s2, channels=P,
                                           reduce_op=bass_isa.ReduceOp.add)
            rs2 = sm_pool.tile([P, 1], f32)
            nc.vector.reciprocal(rs2, s2t)
            o_sb = sm_pool.tile([P, OUT // P], f32)
            nc.vector.tensor_scalar_mul(o_sb, e2, rs2)
            nc.sync.dma_start(out_o[0].rearrange("(p j) -> p j", p=P), o_sb)

    return nc


def _prep_core_inputs(c, inputs):
    f = np.float32
    inp = np.ascontiguousarray(inputs["input"], f).reshape(1, OUT)
    hid = np.ascontiguousarray(inputs["hidden"], f).reshape(1, H)
    enc = np.ascontiguousarray(inputs["encoder_outputs"], f)
    pw = np.ascontiguousarray(inputs["prenet_w"], f)
    pb = np.ascontiguousarray(inputs["prenet_b"], f)
    p2w = np.ascontiguousarray(inputs["prenet2_w"], f)
    p2b = np.ascontiguousarray(inputs["prenet2_b"], f)
    a1 = np.ascontiguousarray(inputs["attn1_w"], f)
    a2 = np.ascontiguousarray(inputs["attn2_w"], f)
    a3 = np.ascontiguousarray(inputs["attn3_w"], f)
    wih = np.ascontiguousarray(inputs["gru_w_ih"], f)
    whh = np.ascontiguousarray(inputs["gru_w_hh"], f)
    bih = np.ascontiguousarray(inputs["gru_b_ih"], f)
    bhh = np.ascontiguousarray(inputs["gru_b_hh"], f)
    lw = np.ascontiguousarray(inputs["lin_w"], f)
    lb = np.ascontiguousarray(inputs["lin_b"], f)

    idx = np.concatenate([
        np.arange(c * GSL, (c + 1) * GSL),
        np.arange(H + c * GSL, H + (c + 1) * GSL),
        np.arange(2 * H + c * GSL, 2 * H + (c + 1) * GSL)])

    import ml_dtypes

    bf = ml_dtypes.bfloat16
    att = (lambda a: a.astype(bf)) if DT_ATT == "bf16" else (lambda a: a)
    wf = (lambda a: a.astype(bf)) if DT_W == "bf16" else (lambda a: a)
    bet = (lambda a: a.astype(bf)) if DT_ATT != "f32" else (lambda a: a)

    C = np.ascontiguousarray
    return {
        "encT": att(C(enc[c * TLOC : (c + 1) * TLOC].T)),
        "a2T": att(C(a2.T)),
        "a1T_s": wf(C(a1.T[:, c * GSL : (c + 1) * GSL])),
        "a3c": bet(C(a3.reshape(H // P, P).T)),
        "pwT": wf(C(pw.T)),
        "p2w": wf(p2w),
        "p2b_c": C(p2b.reshape(H // P, P).T),
        "pb_row": pb.reshape(1, H1),
        "in_c": wf(C(inp.reshape(OUT // P, P).T)),
        "h0c": wf(C(hid.reshape(H // P, P).T)),
        "h0sl": C(hid[:, c * GSL : (c + 1) * GSL]),
        "wihT": wf(C(wih[idx].T)),
        "whhT": wf(C(whh[idx].T)),
        "bih_r": bih[idx].reshape(1, G3),
        "bhh_r": bhh[idx].reshape(1, G3),
        "linT_s": wf(C(lw[:, c * GSL : (c + 1) * GSL].T)),
        "linb8": (lb / NCORES).reshape(1, OUT),
        "ones11": np.ones((1, 1), f),
        "ones_row": np.ones((1, TLOC), f),
    }


def kernel(**inputs):
    from concourse.bass_utils import run_bass_kernel_spmd

    if "nc" not in _CACHE:
        nc = _build()
        if not nc.is_finalized():
            nc.finalize()
        _CACHE["nc"] = nc
    nc = _CACHE["nc"]

    in_maps = [_prep_core_inputs(c, inputs) for c in range(NCORES)]
    trace = os.environ.get("KERNEL_TRACE", "0") == "1"
    res = run_bass_kernel_spmd(nc, in_maps, list(range(NCORES)), trace=trace)
    if trace:
        print("exec_time_ns:", res.exec_time_ns)
        _CACHE["last_result"] = res
    r0 = res.results[0]
    out = r0["out"].reshape(1, OUT)
    h_new = r0["h_new"].reshape(1, 1, H)
    attn_weights = r0["attn_weights"].reshape(1, T)
    return out, h_new, attn_weights


if __name__ == "__main__":
    _build()
    print("build OK")
